# revision 31
# baseline (speedup 1.0000x reference)
"""Correlation (FlowNet-style, max_displacement=4) on 8 TRN2 NeuronCores.

Full inputs x1, x2: [B=8, C=64, H=192, W=192] fp32. Output: [8, 81, 192, 192] fp32.
out[b, di*9+dj, h, w] = mean_c x1[b,c,h,w] * x2pad[b,c,h+di,w+dj]   (di,dj in [0,9))

Strategy: batch-parallel (1 batch per core). Per core the correlation is a
banded Gram matrix on the TensorEngine, computed with COLUMN-GROUP SPLIT
matmuls so PSUM holds only the band (2.37x the useful output instead of
4.74x): for each 16x8 (h,w) output tile, FOUR matmuls (one per 4-h-row pixel
group g, M=32 pixels via tile_position col-group 32g, K=64 channels) each
stream that group's own 12x16 x2 window (N=192) - the per-group window shift
absorbs the band skew at matmul time. Four u-tiles (w-tiles) land in one
2-bank PSUM tile at uniform 256-col slots; one DVE/ACT copy (fp32->bf16,
FD=768) evicts them, engine fixed per h-half so the two evict queues have
independent linear dependency chains. The h-halves live on partitions
0-63 / 64-127 (K=64 each), interleaved so paired matmuls run on disjoint PE
row-groups; the 4 col-groups run on disjoint col-groups (full 128x128 array).

Each strip's band buffer ships as ONE 128-descriptor DMA (18432 B per
descriptor), alternating the sync HWDGE and gpsimd SWDGE rings per strip -
one DMA per strip keeps the Tile semaphore-rebase rendezvous (every ~2 DMAs
per sem, since each DMA bumps its sem by 16) off the critical path; finer
out-DMA splits stalled the whole machine ~5us per strip on those barriers.
Inputs are single SBUF tiles filled by several row-slice DMAs (fine
granularity) across the sync/scalar HWDGE rings + gpsimd SWDGE so strips are
never starved by a monolithic chunk. PE HAM warm-up: 36 dummy matmuls run
during the input-DMA dead time so the real stream starts at 2.4 GHz. x1 is
pre-scaled by 1/64 on the host (exact) so the matmul output is directly the
channel mean. The band is deskewed on the host with a zero-copy strided view.
"""

import sys
import types

import numpy as np
import ml_dtypes

import concourse.bacc as bacc
from concourse import mybir
from concourse.tile import TileContext
from concourse.bass_utils import run_bass_kernel_spmd

B, C, H, W = 8, 64, 192, 192
MAXD = 4
D = 2 * MAXD + 1  # 9
HP, WP = H + 2 * MAXD, W + 2 * MAXD  # 200, 200
TH, TW = 16, 8            # output tile (h, w) -> M = 128
G = 4                     # h-rows per col-group -> 4 groups of M=32
NW = TW + 2 * MAXD        # 16 window cols per u-tile
GR = G + 2 * MAXD         # 12 window rows per col-group
BC = GR * NW              # 192 band cols per u-tile
RC = D * NW               # 144 shipped cols per pixel-row
NSP = H // (2 * TH)       # 6 strips per partition-half
N_WT = W // TW            # 24 w-tiles
NQ = N_WT // 4            # 6 psum-tile groups per (half, strip)
HHALF = H // 2            # 96 rows per partition-half
SLAB = HHALF + 2 * MAXD   # 104 padded x2 rows per half

BF16 = ml_dtypes.bfloat16


def _install_axon_trace_shim():
    """The image's antenv package lacks axon_hooks; run_bass_kernel_spmd
    crashes on import when trace=True. Provide the hook from the boot module
    so tracing works instead of raising."""
    if "antenv.axon_hooks" in sys.modules:
        return
    try:
        import trn_agent_boot.trn_boot as tb

        hook = tb._ntff_profile_via_ctypes("/opt/axon/libaxon_pjrt.so")
    except Exception:
        hook = None
    mod = types.ModuleType("antenv.axon_hooks")
    mod.get_axon_ntff_profile_hook = lambda: hook
    mod.set_axon_ntff_profile_hook = lambda h: None
    sys.modules["antenv.axon_hooks"] = mod


def build_nc():
    nc = bacc.Bacc("TRN2", target_bir_lowering=False, debug=False)
    # x1 arrives pre-tiled: [128, strip, wtile, 128 pixels] - walrus requires
    # the matmul weights AP to have a single free dimension. Pixel order
    # (ph, pw) so [32g:32g+32] is col-group g's 4 h-rows.
    x1s = nc.dram_tensor("x1s", [128, NSP, N_WT, TH * TW], mybir.dt.bfloat16, kind="ExternalInput")
    x2s = nc.dram_tensor("x2s", [128, SLAB, WP], mybir.dt.bfloat16, kind="ExternalInput")
    y = nc.dram_tensor("y", [NSP, 128, 2, N_WT, BC], mybir.dt.bfloat16, kind="ExternalOutput")

    with TileContext(nc) as tc:
        with (
            tc.tile_pool(name="imgs", bufs=1) as imgs,
            tc.tile_pool(name="outs", bufs=3) as outs,
            tc.tile_pool(name="psum", bufs=4, space="PSUM") as psum,
        ):
            # HAM warm-up: the PE clock-gate needs ~3.4us of sustained busy
            # to lift K=4/8 (1.2 GHz) -> 8/8 (2.4 GHz). Dummy matmuls run
            # back-to-back during the otherwise-idle input-DMA window so the
            # real stream starts warm.
            wt = imgs.tile([128, 640], mybir.dt.bfloat16, name="warm", tag="warm")
            nc.vector.memset(wt[:], 0.0)
            wp = psum.tile([128, 1024], mybir.dt.float32, name="warmp", tag="pt")
            for _ in range(36):
                nc.tensor.matmul(wp[:, 0:512], lhsT=wt[:, 0:128],
                                 rhs=wt[:, 128:640], start=True, stop=True)

            # Single input tiles, filled by row-slice DMAs so each strip only
            # waits for its own rows (matmul APs may span slice boundaries -
            # the byte-range dependency tracker joins on both DMAs).
            x2t = imgs.tile([128, SLAB, WP], mybir.dt.bfloat16, name="x2t", tag="x2t")
            x1t = imgs.tile([128, NSP, N_WT, TH * TW], mybir.dt.bfloat16, name="x1t", tag="x1t")
            # strip-0 rows first as per-half 64-descriptor DMAs on three
            # rings, then the rest in ~2-strip slices.
            nc.sync.dma_start(out=x2t[0:64, 0:24], in_=x2s[0:64, 0:24])
            nc.scalar.dma_start(out=x1t[0:64, 0:1], in_=x1s[0:64, 0:1])
            nc.gpsimd.dma_start(out=x2t[64:128, 0:24], in_=x2s[64:128, 0:24])
            nc.gpsimd.dma_start(out=x1t[64:128, 0:1], in_=x1s[64:128, 0:1])
            # Strips 1-2 split into single-strip slices: strip 1 otherwise
            # waits the full 1.64 MB [24:56] slice (~5us later), and that
            # >3.4us PE-idle gap re-throttles the HAM clock to 1.2 GHz.
            nc.sync.dma_start(out=x2t[:, 24:40], in_=x2s[:, 24:40])
            nc.scalar.dma_start(out=x1t[:, 1:2], in_=x1s[:, 1:2])
            nc.sync.dma_start(out=x2t[:, 40:56], in_=x2s[:, 40:56])
            nc.scalar.dma_start(out=x1t[:, 2:3], in_=x1s[:, 2:3])
            nc.sync.dma_start(out=x2t[:, 56:72], in_=x2s[:, 56:72])
            nc.scalar.dma_start(out=x1t[:, 3:4], in_=x1s[:, 3:4])
            nc.sync.dma_start(out=x2t[:, 72:88], in_=x2s[:, 72:88])
            nc.scalar.dma_start(out=x1t[:, 4:5], in_=x1s[:, 4:5])
            nc.sync.dma_start(out=x2t[:, 88:104], in_=x2s[:, 88:104])
            nc.scalar.dma_start(out=x1t[:, 5:6], in_=x1s[:, 5:6])

            for sp in range(NSP):
                if sp == 1:
                    # Warm bridge: four dummy matmuls keyed to the x2[40:56]
                    # slice fire mid-way through the strip-1 input wait,
                    # keeping the PE activity monitor from re-throttling the
                    # clock during the >3.4us gap (4 pool allocations keep
                    # the psum rotation parity intact).
                    for k in range(4):
                        dp = psum.tile([128, 1024], mybir.dt.float32, tag="pt")
                        nc.tensor.matmul(
                            dp[:, 0:400], lhsT=wt[0:64, 0:128],
                            rhs=x2t[0:64, 40 + 2 * k:42 + 2 * k, :],
                            start=True, stop=True)
                h0 = sp * TH  # strip's first row in the slab
                # Band buffer, c-minor: [part, half, wtile, bandcol] so both
                # eviction sides walk contiguous inner runs (a c-major layout
                # for a tighter band ship makes the eviction's inner write
                # stride 96 B, which measured 4x slower on DVE and ACT).
                ybuf = outs.tile([128, 2, N_WT, BC], mybir.dt.bfloat16,
                                 name=f"ybuf_{sp}", tag="ybuf")
                for q in range(NQ):
                    for half in range(2):
                        p0 = 64 * half
                        # 2-bank psum tile; 4 u-tiles at uniform 256-col
                        # slots (192 used each).
                        pt = psum.tile([128, 1024], mybir.dt.float32,
                                       name=f"pt_{sp}_{half}_{q}", tag="pt")
                        for j in range(4):
                            t = 4 * q + j
                            w0 = t * TW
                            for g in range(4):
                                nc.tensor.matmul(
                                    pt[32 * g:32 * g + 32, 256 * j:256 * j + BC],
                                    lhsT=x1t[p0:p0 + 64, sp, t, 32 * g:32 * g + 32],
                                    rhs=x2t[p0:p0 + 64, h0 + G * g:h0 + G * g + GR,
                                            w0:w0 + NW],
                                    start=True, stop=True,
                                    tile_position=(p0, 32 * g),
                                )
                        # Evict all 4 u-tiles' bands with one op; engine
                        # fixed per half (DVE=h0, ACT=h64) so the two evict
                        # FIFOs' dependency chains stay independent (parity
                        # alternation re-coupled them and cost ~8us).
                        src = pt[:].rearrange("p (u c) -> p u c", u=4)[:, :, 0:BC]
                        dst = ybuf[:, half, 4 * q:4 * q + 4, :]
                        if half == 0:
                            nc.vector.tensor_copy(dst, src)
                        else:
                            nc.scalar.copy(dst, src)
                # Ship the whole band buffer in ONE 128-descriptor DMA
                # (18432 B per descriptor), alternating the sync HWDGE and
                # gpsimd SWDGE rings per strip. One DMA per strip keeps the
                # Tile semaphore-rebase rendezvous (every ~2 DMAs per sem,
                # since each DMA bumps its sem by 16) off the critical path -
                # with 4 DMAs/strip those barriers stalled the whole machine
                # ~5us per strip.
                eng = nc.sync if sp % 2 == 0 else nc.gpsimd
                eng.dma_start(out=y[sp], in_=ybuf[:])

    nc.compile()
    return nc


_NC_CACHE = None


def _get_nc():
    global _NC_CACHE
    if _NC_CACHE is None:
        _NC_CACHE = build_nc()
    return _NC_CACHE


def _prep_inputs(x1, x2):
    """Host-side shard prep: scale, pad, split h into partition halves, bf16."""
    in_maps = []
    x1 = np.asarray(x1, dtype=np.float32)
    x2 = np.asarray(x2, dtype=np.float32)
    x1h = (x1 * (1.0 / C)).astype(BF16)
    x2h = x2.astype(BF16)
    for b in range(B):
        # x1: [64, 192, 192] -> pre-tiled [128 = half*64+c, sp, t, ph*TW+pw]
        a = x1h[b].reshape(C, 2, NSP, TH, N_WT, TW)
        a = a.transpose(1, 0, 2, 4, 3, 5).reshape(128, NSP, N_WT, TH * TW)
        # x2: pad to [64, 200, 200], two overlapping 104-row slabs
        p = np.zeros((C, HP, WP), dtype=BF16)
        p[:, MAXD:MAXD + H, MAXD:MAXD + W] = x2h[b]
        s = np.stack([p[:, 0:SLAB, :], p[:, HHALF:HHALF + SLAB, :]], axis=0)
        s = s.reshape(2 * C, SLAB, WP)
        in_maps.append({"x1s": np.ascontiguousarray(a), "x2s": np.ascontiguousarray(s)})
    return in_maps


def _deskew(yb):
    """yb: [NSP, 128, 2, N_WT, BC] fp32 (one batch) -> [81, 192, 192].

    h = half*96 + sp*16 + 4*g + r,  w = 8*t + pw; partition = 32g+8r+pw; the
    value for displacement (di, dj) sits at band col 16*(r+di) + pw + dj.
    """
    s_sp, s_p, s_half, s_t, s_c = yb.strides
    v = np.lib.stride_tricks.as_strided(
        yb,
        shape=(D, D, 2, NSP, G, G, N_WT, TW),
        strides=(NW * s_c, s_c, s_half, s_sp, 32 * s_p,
                 8 * s_p + NW * s_c, s_t, s_p + s_c),
    )
    return np.ascontiguousarray(v).reshape(D * D, H, W)


def kernel(x1, x2):
    _install_axon_trace_shim()
    nc = _get_nc()
    in_maps = _prep_inputs(x1, x2)
    res = run_bass_kernel_spmd(nc, in_maps, core_ids=list(range(B)))
    kernel.last_results = res
    out = np.empty((B, D * D, H, W), dtype=np.float32)
    for b in range(B):
        yb = np.asarray(res.results[b]["y"]).astype(np.float32)
        out[b] = _deskew(yb)
    return out
